# revision 10
# baseline (speedup 1.0000x reference)
"""Chamfer distance (B=16, N=M=4096, D=3) on 8 Trainium2 NeuronCores.

Sharding: data-parallel over batch — 2 batches per core, SPMD.

Per batch, the NxM squared-distance matrix is produced by TensorE as K=15
matmuls using augmented embeddings with an fp16 hi/lo split:
    x~ = [x0,x1,x2, ||x||^2, 1],  y~ = [-2y0,-2y1,-2y2, 1, ||y||^2]
    A_n = [xh, xh, xl],  B_m = [yh, yl, yh]  (each 3x5 = K=15 rows)
so (A.B)[n,m] ~= ||x_n - y_m||^2 at ~fp32 precision (PSUM accumulates fp32).

Since K=15 <= 32, four row tiles are packed concurrently into the PE array
via tile_position row groups (4 quadrant matmuls overlap, ~3-4x TensorE
throughput). Each "quad" covers 4 row tiles; matmuls write 4 PSUM banks per
512-wide m-wave, ScalarE casts each wave PSUM->SBUF fp16 ([128,4,512] per
op), and VectorE consumes the fp16 quad:
  - per-tile row-min in ONE custom DVE instruction (MIN2R_ANT: body
    min(Src0,Src1) over the two tile halves + MIN accumulation to [128,1]),
    replacing the 5-op halving tree + reduce. MIN2R_ANT carries a
    hand-authored 2x_1p uop program (two packed fp16 per port-read, lo/hi
    mins in blocks 0-1, pair-fold + running accumulator in blocks 2-3), so
    it consumes 4 fp16/cycle/lane instead of the 1x custom-op default;
    perf_max=1 on the instruction exposes the 2x table slot to the engine.
  - col-min via a pairwise min tree across the quad then one accumulate
    into the running column accumulator (stock tensor_tensor, 2x fp16).
The tiny epilogue (partition-min of col accumulator, sqrt, mean) runs on
host in fp32.
"""

import numpy as np

import concourse.mybir as mybir
import concourse.tile as tile
from concourse import bacc
from concourse.bass_utils import run_bass_kernel_spmd

B, N, M, D = 16, 4096, 4096, 3
N_CORES = 8
BPC = B // N_CORES  # batches per core
K = 15
NT = N // 128   # 32 row tiles
NQ = NT // 4    # 8 quads of 4 row tiles
NW = M // 512   # 8 m-waves per quad

F16 = mybir.dt.float16
F32 = mybir.dt.float32

BIG = 3.0e38


# ---------------- custom DVE op ---------------- #

_MIN2R = None


def _register_min2r():
    """Register MIN2R_ANT: out = min(in0, in1); accum_out = min(out.min(-1), s0).

    Appended to concourse.dve_ops.OPS at import; the per-NEFF DVE table is
    generated from it at compile time (no firmware change). The uops sha is
    computed from lower() output so the pin is self-consistent.
    """
    global _MIN2R
    if _MIN2R is not None:
        return _MIN2R
    from concourse.dve_ops import (
        OPS,
        CUSTOM_DVE_SPECS,
        DveOp,
        _CUSTOM_DVE_ROW_BASE,
        _SUB_OPCODE_FOR_NAME,
    )
    from concourse.dve_spec import C0, Spec, Src0, Src1, lower, minn
    from concourse.dve_uop import AluOp, DveOpSpec

    name = "MIN2R_ANT"
    if name in _SUB_OPCODE_FOR_NAME:
        _MIN2R = next(o for o in OPS if o.name == name)
        return _MIN2R

    def _ref(in0, in1, s0, s1, imm2):
        b = np.minimum(in0.astype(np.float32), in1.astype(np.float32))
        acc = b.reshape(b.shape[0], -1).min(axis=-1, keepdims=True)
        if isinstance(s0, np.ndarray):
            acc = np.minimum(acc, s0.astype(np.float32))
        else:
            acc = np.minimum(acc, s0)
        return b, acc

    spec = Spec(body=minn(Src0, Src1), accum=AluOp.MIN, accum_init=C0, reference=_ref)
    opcode = _CUSTOM_DVE_ROW_BASE + len(OPS)
    assert opcode < 0x20

    op = _Min2rOp(name, spec, opcode)
    OPS.append(op)
    CUSTOM_DVE_SPECS[name] = spec
    _SUB_OPCODE_FOR_NAME[name] = opcode
    _MIN2R = op
    return op


def _build_min2r_uops_2x():
    """Hand-authored 2x_1p program for MIN2R_ANT.

    Per beat the engine presents two packed fp16 per port: SRC_0/SRC_0_HI
    (in0) and SRC_1/SRC_1_HI (in1). blk0/blk1 compute the lo/hi pairwise
    mins, blk2 folds them, blk3 keeps the running accumulator (CURR_ALU_OUT
    self-reference, A-flop store), blk4..7 propagate the accumulator down
    the bypass/A chain exactly like lower()'s 1x program. The lo/hi results
    ride delay chains 0/1 to the write stage (WR0_LO/WR0_HI re-pack).
    """
    from concourse.dve_uop import (
        DISABLE,
        ENABLE,
        AluInp,
        AluOp,
        DelayInp,
        InpSel,
        OutPath,
        OutSel,
        Trigger,
        UopConfig,
    )

    def mk_inputs(u):
        u.enable_input(InpSel.SRC_0, 1)      # -> blk0 PREV_DELAY_0
        u.enable_input(InpSel.SRC_1, 2)      # -> blk0 PREV_DELAY_1
        u.enable_input(InpSel.CONST_0, 3)    # -> blk0 PREV_DELAY_2 (accum init)
        u.enable_input(InpSel.SRC_0_HI, 4)   # -> blk0 PREV_DELAY_3
        u.enable_input(InpSel.SRC_1_HI, 5)   # -> blk0 PREV_DELAY_4
        return u

    # --- seed state: one beat, loads CONST_0 into the accumulator chain ---
    seed = mk_inputs(UopConfig())
    seed.accum_enabled = ENABLE
    seed.repeat_count = 1
    seed.trigger = (Trigger.COUNT, Trigger.NONE, Trigger.NONE)
    seed.next_uop = (1, 0, 0)
    b = seed.datapath_config
    b[0].enable_alu(AluOp.MIN, AluInp.PREV_DELAY_0, AluInp.PREV_DELAY_1)
    b[0].pass_through_delay(2, 3, 4)
    b[1].enable_alu(AluOp.MIN, AluInp.PREV_DELAY_3, AluInp.PREV_DELAY_4)
    b[1].enable_delay_from_src(DelayInp.PREV_ALU_OUT, 0)
    b[1].pass_through_delay(2)
    b[2].pass_through_alu()
    b[2].pass_through_delay(0, 2)
    b[3].enable_alu(AluOp.BYPASS, AluInp.PREV_DELAY_2, AluInp.PREV_DELAY_2)
    b[3].alu_out_a_enable = ENABLE
    for k in range(4, 8):
        b[k].pass_through_alu()
        b[k].alu_out_a_enable = ENABLE

    # --- steady state: two results + accum per beat until src exhausted ---
    st = mk_inputs(UopConfig())
    st.accum_enabled = ENABLE
    st.require_inp0 = ENABLE
    st.require_inp1 = ENABLE
    st.repeat_count = 0
    st.trigger = (Trigger.SRC_TENSOR_DONE, Trigger.NONE, Trigger.NONE)
    st.next_uop = (0, 0, 0)
    st.enable_output(OutSel.DELAY_0, OutPath.WR0_LO)
    st.enable_output(OutSel.DELAY_1, OutPath.WR0_HI)
    b = st.datapath_config
    b[0].enable_alu(AluOp.MIN, AluInp.PREV_DELAY_0, AluInp.PREV_DELAY_1)
    b[0].pass_through_delay(3, 4)
    b[1].enable_alu(AluOp.MIN, AluInp.PREV_DELAY_3, AluInp.PREV_DELAY_4)
    b[1].enable_delay_from_src(DelayInp.PREV_ALU_OUT, 0)  # lo result
    b[2].enable_alu(AluOp.MIN, AluInp.PREV_ALU_OUT, AluInp.PREV_DELAY_0)
    b[2].pass_through_delay(0)
    b[2].enable_delay_from_src(DelayInp.PREV_ALU_OUT, 1)  # hi result
    b[3].enable_alu(AluOp.MIN, AluInp.CURR_ALU_OUT, AluInp.PREV_ALU_OUT)
    b[3].alu_out_a_enable = ENABLE
    b[3].pass_through_delay(0, 1)
    for k in range(4, 8):
        b[k].pass_through_alu()
        b[k].alu_out_a_enable = ENABLE
        b[k].pass_through_delay(0, 1)

    return [seed, st]


class _Min2rOp:
    """Duck-typed DveOp with a hand-authored 2x_1p perf-mode program."""

    def __init__(self, name, spec, opcode):
        self.name = name
        self.spec = spec
        self.subdim = False
        self.opcode = opcode
        self._cache = {}

    def compile(self, ver):
        if ver in self._cache:
            return self._cache[ver]
        from concourse.dve_spec import lower
        from concourse.dve_uop import DveOpSpec

        uops = lower(self.spec, ver=ver)
        uops_2x = _build_min2r_uops_2x()
        assert len(uops_2x) == len(uops), (len(uops_2x), len(uops))
        result = DveOpSpec(
            name=self.name,
            opcode=self.opcode,
            uops=uops,
            uops_2x=uops_2x,
            perf_max=1,
            rd1_en=True,
        )
        result.validate(ver)
        self._cache[ver] = result
        return result


def _emit_min2r(nc, op, *, out, in0, in1, s0, accum_out):
    """nc.vector._custom_dve equivalent that also sets perf_max=1 so the
    engine can reach the 2x_1p table slot (byte-36[7:6])."""
    import concourse.bass_isa as bass_isa
    import concourse.mybir as _mybir

    if op.name not in nc.m.ant_custom_dve_ops:
        nc.m.ant_custom_dve_ops = sorted({*nc.m.ant_custom_dve_ops, op.name})
    v = nc.vector
    ins = [
        v.lower_ap(in0, for_isa=True, opt=True),
        v.lower_ap(in1, for_isa=True, opt=True),
        _mybir.ImmediateValue(dtype=_mybir.dt.float32, value=float(s0)),
        _mybir.ImmediateValue(dtype=_mybir.dt.float32, value=0.0),
    ]
    outs = [
        v.lower_ap(out, for_isa=True, opt=True),
        v.lower_ap(accum_out, for_isa=True),
    ]
    isa_opcode = nc.isa.Opcode[
        f"NEURON_ISA_TPB_OPCODE_CUSTOM_DVE_ANT_{bass_isa.CustomDveShape.TTSS.slot()}"
    ].value
    return v.add_instruction(
        bass_isa.InstCustomDveAnt(
            name=nc.get_next_instruction_name(),
            op_name=op.name,
            rd1_en=True,
            subdim=0,
            imm2=0.0,
            shape=bass_isa.CustomDveShape.TTSS,
            row=op.opcode,
            perf_max=1,
            isa_opcode=isa_opcode,
            ins=ins,
            outs=outs,
        )
    )


# ---------------- host packing ---------------- #

def host_pack(x: np.ndarray, y: np.ndarray):
    """x, y: [B, N, 3] f32 -> Aq [B, NQ, 128, 512] f16, Brep [B, 128, M] f16.

    Aq[b, q, 32*i + k, n] = lhsT row k of row-tile 4q+i (K=15 rows used per
    32-row group; rows 15..31 of each group unused). Brep replicates the
    15 y~ rows into all four 32-partition groups so each row-group matmul
    streams its own copy.
    """
    xd = x.astype(np.float64)
    yd = y.astype(np.float64)
    ones_x = np.ones((*xd.shape[:2], 1))
    ones_y = np.ones((*yd.shape[:2], 1))
    xt = np.concatenate([xd, (xd * xd).sum(-1, keepdims=True), ones_x], axis=-1)
    yt = np.concatenate([-2.0 * yd, ones_y, (yd * yd).sum(-1, keepdims=True)], axis=-1)
    xh = xt.astype(np.float16)
    xl = (xt - xh.astype(np.float64)).astype(np.float16)
    yh = yt.astype(np.float16)
    yl = (yt - yh.astype(np.float64)).astype(np.float16)
    A = np.concatenate([xh, xh, xl], axis=-1)   # [B, N, 15]
    Bm = np.concatenate([yh, yl, yh], axis=-1)  # [B, M, 15]

    Aq = np.zeros((B, NQ, 128, 512), np.float16)
    At = A.transpose(0, 2, 1)  # [B, 15, N]
    for q in range(NQ):
        for i in range(4):
            r = 4 * q + i
            Aq[:, q, 32 * i : 32 * i + K, :128] = At[:, :, 128 * r : 128 * (r + 1)]
    Brep = np.zeros((B, 128, M), np.float16)
    Bt = Bm.transpose(0, 2, 1)  # [B, 15, M]
    for i in range(4):
        Brep[:, 32 * i : 32 * i + K, :] = Bt
    return Aq, Brep


# ---------------- device kernel ---------------- #

def build_nc(bpc: int = BPC, reps: int = 1):
    op = _register_min2r()
    nc = bacc.Bacc("TRN2", target_bir_lowering=False, debug=False)
    a_d = nc.dram_tensor("a", [bpc, NQ, 128, 512], F16, kind="ExternalInput")
    b_d = nc.dram_tensor("b", [bpc, 128, M], F16, kind="ExternalInput")
    rm_d = nc.dram_tensor("rowmins", [bpc, 128, NT], F16, kind="ExternalOutput")
    cm_d = nc.dram_tensor("colmins", [bpc, 128, M], F16, kind="ExternalOutput")

    with tile.TileContext(nc) as tc:
        with (
            tc.tile_pool(name="ab", bufs=2) as ab_pool,
            tc.tile_pool(name="quad", bufs=2) as quad_pool,
            tc.tile_pool(name="acc", bufs=2) as acc_pool,
            tc.tile_pool(name="small", bufs=2) as small_pool,
            tc.tile_pool(name="psum", bufs=2, space="PSUM") as psum_pool,
        ):
            for rep in range(reps):
                for bi in range(bpc):
                    a_s = ab_pool.tile([128, NQ, 512], F16, tag="a")
                    b_s = ab_pool.tile([128, M], F16, tag="b")
                    nc.sync.dma_start(
                        a_s[:], a_d.ap()[bi].rearrange("q p f -> p q f")
                    )
                    nc.sync.dma_start(b_s[:], b_d.ap()[bi])
                    colacc = acc_pool.tile([128, M], F16)
                    rowm = small_pool.tile([128, NT], F16)
                    for q in range(NQ):
                        t16 = quad_pool.tile([128, 4, M], F16, tag="t16")
                        for w in range(NW):
                            ps = psum_pool.tile([128, 4, 512], F32, tag="ps")
                            for i in range(4):
                                nc.tensor.matmul(
                                    ps[:, i, :],
                                    a_s[32 * i : 32 * i + K, q, :128],
                                    b_s[32 * i : 32 * i + K, 512 * w : 512 * (w + 1)],
                                    start=True,
                                    stop=True,
                                    tile_position=(32 * i, 0),
                                )
                            nc.scalar.copy(
                                t16[:, :, 512 * w : 512 * (w + 1)], ps[:]
                            )
                        # per-tile rowmin: one fused custom op each
                        u = quad_pool.tile([128, 4, M // 2], F16, tag="u")
                        for i in range(4):
                            _emit_min2r(
                                nc,
                                op,
                                out=u[:, i, :],
                                in0=t16[:, i, : M // 2],
                                in1=t16[:, i, M // 2 :],
                                s0=BIG,
                                accum_out=rowm[:, 4 * q + i : 4 * q + i + 1],
                            )
                        # colmin: quad pair tree, then accumulate
                        v = quad_pool.tile([128, 2, M], F16, tag="v")
                        nc.vector.tensor_tensor(
                            v[:], t16[:, 0:2, :], t16[:, 2:4, :], mybir.AluOpType.min
                        )
                        if q == 0:
                            nc.vector.tensor_tensor(
                                colacc[:], v[:, 0, :], v[:, 1, :], mybir.AluOpType.min
                            )
                        else:
                            nc.vector.tensor_tensor(
                                v[:, 0, :], v[:, 0, :], v[:, 1, :], mybir.AluOpType.min
                            )
                            nc.vector.tensor_tensor(
                                colacc[:], v[:, 0, :], colacc[:], mybir.AluOpType.min
                            )
                    nc.sync.dma_start(rm_d.ap()[bi], rowm[:])
                    nc.sync.dma_start(cm_d.ap()[bi], colacc[:])
    nc.compile()
    return nc


# ---------------- host epilogue ---------------- #

def host_finish(rowmins: np.ndarray, colmins: np.ndarray):
    """rowmins [bpc,128,NT] f16, colmins [bpc,128,m] f16 -> cost [bpc] f32."""
    rm = np.clip(rowmins.astype(np.float32), 0.0, None)
    cm = np.clip(colmins.astype(np.float32).min(axis=1), 0.0, None)
    d1 = np.sqrt(rm.reshape(rm.shape[0], -1)).mean(axis=1)
    d2 = np.sqrt(cm).mean(axis=1)
    return ((d1 + d2) * 0.5).astype(np.float32)


_RUN_KWARGS = {}
_NC_CACHE = None


def _get_nc():
    global _NC_CACHE
    if _NC_CACHE is None:
        _NC_CACHE = build_nc()
    return _NC_CACHE


def kernel(x: np.ndarray, y: np.ndarray) -> np.ndarray:
    x = np.asarray(x, dtype=np.float32)
    y = np.asarray(y, dtype=np.float32)
    Aq, Brep = host_pack(x, y)
    nc = _get_nc()
    in_maps = [
        {"a": Aq[c * BPC : (c + 1) * BPC], "b": Brep[c * BPC : (c + 1) * BPC]}
        for c in range(N_CORES)
    ]
    res = run_bass_kernel_spmd(nc, in_maps, core_ids=list(range(N_CORES)), **_RUN_KWARGS)
    out = np.empty((B,), dtype=np.float32)
    for c in range(N_CORES):
        out[c * BPC : (c + 1) * BPC] = host_finish(
            res.results[c]["rowmins"], res.results[c]["colmins"]
        )
    return out
